# revision 1
# baseline (speedup 1.0000x reference)
"""BitLinear (ternary-weight linear) kernel for Trainium2, 8 NeuronCores.

Computation:  out = x @ (w_ternary * scale)^T
  where scale = max(mean(|weight|), 1e-5)
        w_ternary = clip(round(weight / scale), -1, 1)  in {-1, 0, 1}

Strategy:
  - Host: quantize the 4 MB weight (tiny, elementwise) and pre-transpose it
    to wT [in, out]; scale is passed as a [1,1] tensor and applied by the
    scalar engine during the PSUM->SBUF output copy.
  - Device (data-parallel over the batch dim, 1 batch row per core):
    out_b = x_b @ wT with ternary +/-1 weights, fp32r matmuls (full PE rate
    at free dim >= 256, ~13 mantissa bits so +/-1 weights are exact and x
    carries ~6e-5 relative rounding).
    Per 128-row block of x_b: DMA the natural [128, 1024] tile, PE-transpose
    its 8 column slices (contraction dim must sit on partitions), DVE-copy
    the transposed slices out of PSUM, then 16 accumulating matmuls
    (lhsT = xT tile, rhs = wT slice) produce PSUM [128 s, 1024 o] which the
    scalar engine copies out with the scale applied, and DMA stores.
"""

import numpy as np

B, S, IN, OUT = 8, 8192, 1024, 1024
N_CORES = 8
P = 128
S_BLOCKS = S // P  # 64
K_TILES = IN // P  # 8
EPS = 1e-5

_compiled = None


def _build():
    import concourse.bacc as bacc
    import concourse.mybir as mybir
    import concourse.tile as tile

    R = mybir.dt.float32r
    F32 = mybir.dt.float32

    nc = bacc.Bacc()
    x = nc.declare_dram_parameter("x", [S, IN], R, isOutput=False)
    wt = nc.declare_dram_parameter("wt", [IN, OUT], R, isOutput=False)
    ident = nc.declare_dram_parameter("ident", [P, P], R, isOutput=False)
    scale_t = nc.declare_dram_parameter("scale", [1, 1], F32, isOutput=False)
    out = nc.declare_dram_parameter("out", [S, OUT], F32, isOutput=True)

    with tile.TileContext(nc) as tc:
        with (
            tc.tile_pool(name="const", bufs=1) as constp,
            tc.tile_pool(name="xn", bufs=3) as xnp,
            tc.tile_pool(name="xt", bufs=6) as xtp,
            tc.tile_pool(name="outp", bufs=3) as outp,
            tc.tile_pool(name="pst", bufs=4, space="PSUM") as pst,
            tc.tile_pool(name="pso", bufs=4, space="PSUM") as pso,
        ):
            ident_sb = constp.tile([P, P], R)
            nc.sync.dma_start(out=ident_sb, in_=ident[:])

            xn_tiles = {}

            def load_xn(b, halves=1):
                if b < S_BLOCKS and b not in xn_tiles:
                    t = xnp.tile([P, IN], R, tag="xn", name=f"xn_{b}")
                    hw = IN // halves
                    for i in range(halves):
                        nc.sync.dma_start(
                            out=t[:, i * hw:(i + 1) * hw],
                            in_=x[b * P:(b + 1) * P, i * hw:(i + 1) * hw],
                        )
                    xn_tiles[b] = t

            load_xn(0, halves=2)

            # Transposed ternary weight resident in SBUF: [128, k, 1024].
            # All startup DMAs go on the one Sync ring in priority order
            # (ident, x block 0, then weight k-slices interleaved with the
            # next x block) — a single ring drains strictly in order, so the
            # first transposes and first matmuls see their data earliest.
            wt_sb = constp.tile([P, K_TILES, OUT], R)
            wt_r = wt[:].rearrange("(a p) o -> p a o", p=P)
            for k in range(4):
                nc.sync.dma_start(
                    out=wt_sb[:, k:k + 1, :], in_=wt_r[:, k:k + 1, :]
                )
            load_xn(1)
            for k in range(4, K_TILES):
                nc.sync.dma_start(
                    out=wt_sb[:, k:k + 1, :], in_=wt_r[:, k:k + 1, :]
                )

            # scale broadcast to all 128 partitions for the scaled copy
            # (after the weight DMAs: the 128-way replicated write is slow
            # and must not delay the k=0 weight slice)
            scale_sb = constp.tile([P, 1], F32)
            nc.gpsimd.dma_start(
                out=scale_sb, in_=scale_t[:].to_broadcast((P, 1))
            )

            # Software-pipelined emission: the PE-transposes (+DVE copies)
            # for block b+1 are emitted BEFORE block b's matmuls, so the
            # copies complete during the 3.6us matmul phase and the next
            # block's first matmul never stalls on its transposed operand.
            def emit_transposes(b):
                # PE-transpose the 8 [128,128] column slices; pack 4 per
                # PSUM bank so 8 transposes only hold 2 banks.
                xn_sb = xn_tiles.pop(b)
                load_xn(b + 2)
                pts = [pst.tile([P, 4, P], R, tag="pst", name=f"pt{b}_{i}")
                       for i in range(2)]
                xts = [xtp.tile([P, 4, P], R, tag="xt4", name=f"xt{b}_{i}")
                       for i in range(2)]
                for i in range(2):
                    for j in range(4):
                        k = 4 * i + j
                        nc.tensor.transpose(
                            pts[i][:, j, :],
                            xn_sb[:, k * P:(k + 1) * P],
                            ident_sb,
                        )
                    nc.vector.tensor_copy(xts[i], pts[i])
                return xts

            xts_cur = emit_transposes(0)
            for b in range(S_BLOCKS):
                xts_next = (emit_transposes(b + 1)
                            if b + 1 < S_BLOCKS else None)

                # h-outer: finish the o-half-0 accumulation first so its
                # scaled copy + store overlap the o-half-1 matmuls; per-
                # element k order is unchanged, so numerics are identical.
                out_sb = outp.tile([P, OUT], F32)
                for h in range(2):
                    po_h = pso.tile([P, 512], F32, tag="pso",
                                    name=f"po{b}_{h}")
                    for k in range(K_TILES):
                        nc.tensor.matmul(
                            po_h,
                            lhsT=xts_cur[k // 4][:, k % 4, :],
                            rhs=wt_sb[:, k, h * 512:(h + 1) * 512],
                            start=(k == 0),
                            stop=(k == K_TILES - 1),
                        )
                    # last block's final half drains in 256-wide chunks
                    # so the closing copy->store chain is shorter
                    n_chunks = 2 if (b == S_BLOCKS - 1 and h == 1) else 1
                    cw = 512 // n_chunks
                    for c in range(n_chunks):
                        lo = h * 512 + c * cw
                        nc.scalar.activation(
                            out_sb[:, lo:lo + cw],
                            po_h[:, c * cw:(c + 1) * cw],
                            mybir.ActivationFunctionType.Copy,
                            scale=scale_sb[:, 0:1],
                        )
                        nc.sync.dma_start(
                            out=out[b * P:(b + 1) * P, lo:lo + cw],
                            in_=out_sb[:, lo:lo + cw],
                        )
                xts_cur = xts_next
    nc.finalize()
    return nc


def _get_compiled():
    global _compiled
    if _compiled is None:
        _compiled = _build()
    return _compiled


def quantize_host(weight: np.ndarray):
    """Mirror of the reference ste_quantize, done on host in fp32.

    The mean is computed in float64 then rounded to fp32 so it tracks the
    true mean more closely than any fp32 summation order.
    """
    scale = np.float32(max(np.mean(np.abs(weight), dtype=np.float64), EPS))
    w_t = np.clip(np.round(weight / scale), -1.0, 1.0).astype(np.float32)
    return w_t, scale


def kernel(x: np.ndarray, weight: np.ndarray) -> np.ndarray:
    from concourse.bass_utils import run_bass_kernel_spmd

    x = np.asarray(x, dtype=np.float32)
    weight = np.asarray(weight, dtype=np.float32)
    assert x.shape == (B, S, IN) and weight.shape == (OUT, IN)
    w_t, scale = quantize_host(weight)
    wt_T = np.ascontiguousarray(w_t.T)  # [in, out]
    ident = np.eye(P, dtype=np.float32)
    scale_arr = np.array([[scale]], dtype=np.float32)

    nc = _get_compiled()
    in_maps = [
        {"x": np.ascontiguousarray(x[c]), "wt": wt_T, "ident": ident,
         "scale": scale_arr}
        for c in range(N_CORES)
    ]
    res = run_bass_kernel_spmd(nc, in_maps, core_ids=list(range(N_CORES)))
    return np.stack([res.results[c]["out"] for c in range(N_CORES)], axis=0)



# revision 2
# speedup vs baseline: 1.2731x; 1.2731x over previous
"""BitLinear (ternary-weight linear) kernel for Trainium2, 8 NeuronCores.

Computation:  out = x @ (w_ternary * scale)^T
  where scale = max(mean(|weight|), 1e-5)
        w_ternary = clip(round(weight / scale), -1.0, 1.0)  in {-1, 0, 1}

Strategy (data-parallel over batch, 1 batch row per core):
  - Host: quantize the weight to ternary (bit-exact mirror of the jnp
    reference), and split x*(64*scale) into an fp8-e4m3 hi/lo pair
    (x64 = hi + lo exactly to ~2^-9 relative).  The 64*scale folding keeps
    the fp8 values ~N(0,1), far from the e4m3 subnormal range; the exact
    power-of-two 2^-6 is unfolded on device in the output copy.
    Both planes are packed k-major per 128-row block so the device needs
    no transposes or casts at all:
      xp[b, kp, g, s] = plane_g[b*128+s, ks_g*128+kp]
    with g = 0..7 the hi plane (ks = g) and g = 8..15 the lo plane
    (ks = g-8).  Each block slab is a fully contiguous 256 KB DMA.
  - Device: pure fp8 DoubleRow matmuls (2 k-slices per instruction, 0.5
    cycles/row): per 128-row block, 8 stationary lhsT pair-tiles
    [128, 2, 128], each streamed against both 512-wide output halves of
    the un-duplicated fp8 weight, accumulating K_eff = 2048 (hi+lo) into
    two PSUM banks.  The scalar engine copies PSUM -> SBUF bf16 with the
    exact 1/64 scale, and the result DMAs out as bf16 (upcast to fp32 on
    host).  rel err vs the fp32 reference ~3.0e-3.
"""

import numpy as np
import ml_dtypes

B, S, IN, OUT = 8, 8192, 1024, 1024
N_CORES = 8
P = 128
S_BLOCKS = S // P    # 64
K_TILES = IN // P    # 8
G = 2 * K_TILES      # 16 packed k-groups: hi plane then lo plane
EPS = 1e-5

F8 = ml_dtypes.float8_e4m3
BF16 = ml_dtypes.bfloat16

_compiled = None


def _build():
    import concourse.bacc as bacc
    import concourse.mybir as mybir
    import concourse.tile as tile

    F8D = mybir.dt.float8e4
    F32 = mybir.dt.float32
    BF = mybir.dt.bfloat16
    DR = mybir.MatmulPerfMode.DoubleRow

    nc = bacc.Bacc()
    xp = nc.declare_dram_parameter("xp", [S, G * P], F8D, isOutput=False)
    wt = nc.declare_dram_parameter("wt", [P, K_TILES * OUT], F8D, isOutput=False)
    out = nc.declare_dram_parameter("out", [S, OUT], BF, isOutput=True)

    with tile.TileContext(nc) as tc:
        with (
            tc.tile_pool(name="const", bufs=1) as constp,
            tc.tile_pool(name="xn", bufs=12) as xnp,
            tc.tile_pool(name="outp", bufs=4) as outp,
            tc.tile_pool(name="pso", bufs=6, space="PSUM") as pso,
        ):
            # fp8 weight resident in SBUF, [128 kp, 8 ks, 1024 o]; 8 chunk
            # DMAs on the gpsimd ring so the first matmuls (which need only
            # ks 0,1) start after ~0.8us, in parallel with x block 0 on the
            # sync ring.
            wt_sb = constp.tile([P, K_TILES, OUT], F8D)
            for k in range(K_TILES):
                nc.gpsimd.dma_start(
                    out=wt_sb[:, k:k + 1, :],
                    in_=wt[:, k * OUT:(k + 1) * OUT],
                )

            for b in range(S_BLOCKS):
                xt = xnp.tile([P, G, P], F8D, tag="xn", name=f"xn_{b}")
                nc.sync.dma_start(
                    out=xt, in_=xp[b * P:(b + 1) * P, :]
                )

                po = [pso.tile([P, 512], F32, tag="pso", name=f"po{b}_{h}")
                      for h in range(2)]
                # 8 stationary pair-tiles; each serves both output halves
                # back-to-back so weight loads are halved.  t = 0..3 is the
                # hi plane, t = 4..7 the lo plane, both against W k-slices
                # (2t mod 8, 2t+1 mod 8).
                for t in range(K_TILES):
                    lhsT = xt[:, 2 * t:2 * t + 2, :]
                    wk = (2 * t) % K_TILES
                    for h in range(2):
                        nc.tensor.matmul(
                            po[h],
                            lhsT=lhsT,
                            rhs=wt_sb[:, wk:wk + 2, h * 512:(h + 1) * 512],
                            start=(t == 0),
                            stop=(t == K_TILES - 1),
                            perf_mode=DR,
                        )

                ob = outp.tile([P, OUT], BF, tag="ob", name=f"ob_{b}")
                for h in range(2):
                    nc.scalar.activation(
                        ob[:, h * 512:(h + 1) * 512],
                        po[h],
                        mybir.ActivationFunctionType.Copy,
                        scale=1.0 / 64.0,
                    )
                    nc.gpsimd.dma_start(
                        out=out[b * P:(b + 1) * P, h * 512:(h + 1) * 512],
                        in_=ob[:, h * 512:(h + 1) * 512],
                    )
    nc.finalize()
    return nc


def _get_compiled():
    global _compiled
    if _compiled is None:
        _compiled = _build()
    return _compiled


def quantize_host(weight: np.ndarray):
    """Mirror of the reference ste_quantize, done on host in fp32.

    The mean is computed in float64 then rounded to fp32 so it tracks the
    true mean more closely than any fp32 summation order.
    """
    scale = np.float32(max(np.mean(np.abs(weight), dtype=np.float64), EPS))
    w_t = np.clip(np.round(weight / scale), -1.0, 1.0).astype(np.float32)
    return w_t, scale


def pack_weight(w_t: np.ndarray) -> np.ndarray:
    """Ternary weight [out, in] -> fp8 [128 kp, 8 ks * 1024 o]."""
    wt_T = np.ascontiguousarray(w_t.T)  # [in, out]
    return np.ascontiguousarray(
        wt_T.reshape(K_TILES, P, OUT).transpose(1, 0, 2)
    ).astype(F8).reshape(P, K_TILES * OUT)


def pack_x_core(xc: np.ndarray, c64: np.float32) -> np.ndarray:
    """One core's x [S, IN] fp32 -> packed fp8 hi/lo [S, G*P]."""
    xs = xc * c64
    hi = xs.astype(F8)
    lo = (xs - hi.astype(np.float32)).astype(F8)
    xp = np.empty((S_BLOCKS, P, G, P), dtype=F8)
    xp[:, :, 0:K_TILES, :] = hi.reshape(
        S_BLOCKS, P, K_TILES, P).transpose(0, 3, 2, 1)
    xp[:, :, K_TILES:G, :] = lo.reshape(
        S_BLOCKS, P, K_TILES, P).transpose(0, 3, 2, 1)
    return xp.reshape(S, G * P)


def make_in_maps(x: np.ndarray, weight: np.ndarray):
    x = np.asarray(x, dtype=np.float32)
    weight = np.asarray(weight, dtype=np.float32)
    assert x.shape == (B, S, IN) and weight.shape == (OUT, IN)
    w_t, scale = quantize_host(weight)
    wt8 = pack_weight(w_t)
    c64 = np.float32(64.0) * scale
    return [
        {"xp": pack_x_core(x[c], c64), "wt": wt8}
        for c in range(N_CORES)
    ]


def kernel(x: np.ndarray, weight: np.ndarray) -> np.ndarray:
    from concourse.bass_utils import run_bass_kernel_spmd

    in_maps = make_in_maps(x, weight)
    nc = _get_compiled()
    res = run_bass_kernel_spmd(nc, in_maps, core_ids=list(range(N_CORES)))
    return np.stack(
        [res.results[c]["out"].astype(np.float32) for c in range(N_CORES)],
        axis=0,
    )


# revision 6
# speedup vs baseline: 1.6119x; 1.2661x over previous
"""BitLinear (ternary-weight linear) kernel for Trainium2, 8 NeuronCores.

Computation:  out = x @ (w_ternary * scale)^T
  where scale = max(mean(|weight|), 1e-5)
        w_ternary = clip(round(weight / scale), -1.0, 1.0)  in {-1, 0, 1}

Strategy (data-parallel over batch, 1 batch row per core):
  - Host: quantize the weight to ternary (bit-exact mirror of the jnp
    reference), and split x*(64*scale) into an fp8-e4m3 hi/lo pair
    (x64 = hi + lo exactly to ~2^-9 relative).  The 64*scale folding keeps
    the fp8 values ~N(0,1), far from the e4m3 subnormal range; the exact
    power-of-two 2^-6 is unfolded on device in the output copy.
    Both planes are packed k-major per 128-row block so the device needs
    no transposes or casts at all:
      xp[b, kp, g, s] = plane_g[b*128+s, ks_g*128+kp]
    with g = 0..7 the hi plane (ks = g) and g = 8..15 the lo plane
    (ks = g-8).  Each block slab is a fully contiguous 256 KB DMA.
  - Device: pure fp8 DoubleRow matmuls (2 k-slices per instruction, 0.5
    cycles/row): per 128-row block, 8 stationary lhsT pair-tiles
    [128, 2, 128], each streamed against both 512-wide output halves of
    the un-duplicated fp8 weight, accumulating K_eff = 2048 (hi+lo) into
    two PSUM banks.  The scalar engine copies PSUM -> SBUF bf16 with the
    exact 1/64 scale, and the result DMAs out as bf16 (upcast to fp32 on
    host).  rel err vs the fp32 reference ~3.0e-3.
"""

import numpy as np
import ml_dtypes

B, S, IN, OUT = 8, 8192, 1024, 1024
N_CORES = 8
P = 128
S_BLOCKS = S // P    # 64
K_TILES = IN // P    # 8
LO_SLICES = [4, 5, 6, 7]   # k-slices that get the fp8 lo correction
G = K_TILES + len(LO_SLICES)  # 12 packed k-groups: hi plane + partial lo
EPS = 1e-5

F8 = ml_dtypes.float8_e4m3
BF16 = ml_dtypes.bfloat16

_compiled = None


def _build():
    import concourse.bacc as bacc
    import concourse.mybir as mybir
    import concourse.tile as tile

    F8D = mybir.dt.float8e4
    F32 = mybir.dt.float32
    BF = mybir.dt.bfloat16
    DR = mybir.MatmulPerfMode.DoubleRow

    nc = bacc.Bacc()
    xp = nc.declare_dram_parameter("xp", [S, G * P], F8D, isOutput=False)
    wt = nc.declare_dram_parameter("wt", [P, K_TILES * OUT], F8D, isOutput=False)
    out = nc.declare_dram_parameter("out", [S, OUT], BF, isOutput=True)

    # DR pair schedule: (x-group pair start, weight k-slice pair start).
    # Groups 0..7 are the hi plane (ks = g); groups 8..11 are the lo plane
    # for k-slices LO_SLICES = 4..7.
    pairs = [(0, 0), (2, 2), (4, 4), (6, 6), (8, 4), (10, 6)]

    with tile.TileContext(nc) as tc:
        with (
            tc.tile_pool(name="const", bufs=1) as constp,
            tc.tile_pool(name="xn", bufs=5) as xnp,
            tc.tile_pool(name="outp", bufs=4) as outp,
            tc.tile_pool(name="pso", bufs=6, space="PSUM") as pso,
        ):
            # fp8 weight resident in SBUF, [128 kp, 8 ks, 1024 o]; 8 chunk
            # DMAs on the gpsimd ring so the first matmuls (which need only
            # ks 0,1) start after ~0.8us, in parallel with x block 0 on the
            # sync ring.
            wt_sb = constp.tile([P, K_TILES, OUT], F8D)
            for k in range(K_TILES):
                nc.gpsimd.dma_start(
                    out=wt_sb[:, k:k + 1, :],
                    in_=wt[:, k * OUT:(k + 1) * OUT],
                )

            for b in range(S_BLOCKS):
                xt = xnp.tile([P, G, P], F8D, tag="xn", name=f"xn_{b}")
                nc.sync.dma_start(
                    out=xt, in_=xp[b * P:(b + 1) * P, :]
                )

                po = [pso.tile([P, 512], F32, tag="pso", name=f"po{b}_{h}")
                      for h in range(2)]
                # 6 stationary pair-tiles; each serves both output halves
                # back-to-back so weight loads are halved.
                for t, (xg, wk) in enumerate(pairs):
                    lhsT = xt[:, xg:xg + 2, :]
                    for h in range(2):
                        nc.tensor.matmul(
                            po[h],
                            lhsT=lhsT,
                            rhs=wt_sb[:, wk:wk + 2, h * 512:(h + 1) * 512],
                            start=(t == 0),
                            stop=(t == len(pairs) - 1),
                            perf_mode=DR,
                        )

                ob = outp.tile([P, OUT], BF, tag="ob", name=f"ob_{b}")
                for h in range(2):
                    nc.scalar.activation(
                        ob[:, h * 512:(h + 1) * 512],
                        po[h],
                        mybir.ActivationFunctionType.Copy,
                        scale=1.0 / 64.0,
                    )
                nc.gpsimd.dma_start(
                    out=out[b * P:(b + 1) * P, :], in_=ob
                )
    nc.finalize()
    return nc


def _get_compiled():
    global _compiled
    if _compiled is None:
        _compiled = _build()
    return _compiled


def quantize_host(weight: np.ndarray):
    """Mirror of the reference ste_quantize, done on host in fp32.

    The mean is computed in float64 then rounded to fp32 so it tracks the
    true mean more closely than any fp32 summation order.
    """
    scale = np.float32(max(np.mean(np.abs(weight), dtype=np.float64), EPS))
    w_t = np.clip(np.round(weight / scale), -1.0, 1.0).astype(np.float32)
    return w_t, scale


def pack_weight(w_t: np.ndarray) -> np.ndarray:
    """Ternary weight [out, in] -> fp8 [128 kp, 8 ks * 1024 o]."""
    wt_T = np.ascontiguousarray(w_t.T)  # [in, out]
    return np.ascontiguousarray(
        wt_T.reshape(K_TILES, P, OUT).transpose(1, 0, 2)
    ).astype(F8).reshape(P, K_TILES * OUT)


def pack_x_core(xc: np.ndarray, c64: np.float32) -> np.ndarray:
    """One core's x [S, IN] fp32 -> packed fp8 hi + partial lo [S, G*P]."""
    xs = xc * c64
    hi = xs.astype(F8)
    lo_cols = np.concatenate(
        [np.arange(sl * P, (sl + 1) * P) for sl in LO_SLICES])
    lo = (xs[:, lo_cols] - hi.astype(np.float32)[:, lo_cols]).astype(F8)
    xp = np.empty((S_BLOCKS, P, G, P), dtype=F8)
    xp[:, :, 0:K_TILES, :] = hi.reshape(
        S_BLOCKS, P, K_TILES, P).transpose(0, 3, 2, 1)
    xp[:, :, K_TILES:G, :] = lo.reshape(
        S_BLOCKS, P, len(LO_SLICES), P).transpose(0, 3, 2, 1)
    return xp.reshape(S, G * P)


def make_in_maps(x: np.ndarray, weight: np.ndarray):
    x = np.asarray(x, dtype=np.float32)
    weight = np.asarray(weight, dtype=np.float32)
    assert x.shape == (B, S, IN) and weight.shape == (OUT, IN)
    w_t, scale = quantize_host(weight)
    wt8 = pack_weight(w_t)
    c64 = np.float32(64.0) * scale
    return [
        {"xp": pack_x_core(x[c], c64), "wt": wt8}
        for c in range(N_CORES)
    ]


def kernel(x: np.ndarray, weight: np.ndarray) -> np.ndarray:
    from concourse.bass_utils import run_bass_kernel_spmd

    in_maps = make_in_maps(x, weight)
    nc = _get_compiled()
    res = run_bass_kernel_spmd(nc, in_maps, core_ids=list(range(N_CORES)))
    return np.stack(
        [res.results[c]["out"].astype(np.float32) for c in range(N_CORES)],
        axis=0,
    )


# revision 11
# speedup vs baseline: 1.6743x; 1.0387x over previous
"""BitLinear (ternary-weight linear) kernel for Trainium2, 8 NeuronCores.

Computation:  out = x @ (w_ternary * scale)^T
  where scale = max(mean(|weight|), 1e-5)
        w_ternary = clip(round(weight / scale), -1.0, 1.0)  in {-1, 0, 1}

Strategy (data-parallel over batch, 1 batch row per core):
  - Host: quantize the weight to ternary (bit-exact mirror of the jnp
    reference), and split x*(64*scale) into an fp8-e4m3 hi/lo pair
    (x64 = hi + lo exactly to ~2^-9 relative).  The 64*scale folding keeps
    the fp8 values ~N(0,1), far from the e4m3 subnormal range; the exact
    power-of-two 2^-6 is unfolded on device in the output copy.
    Both planes are packed k-major per 128-row block so the device needs
    no transposes or casts at all:
      xp[b, kp, g, s] = plane_g[b*128+s, ks_g*128+kp]
    with g = 0..7 the hi plane (ks = g) and g = 8..15 the lo plane
    (ks = g-8).  Each block slab is a fully contiguous 256 KB DMA.
  - Device: pure fp8 DoubleRow matmuls (2 k-slices per instruction, 0.5
    cycles/row): per 128-row block, 8 stationary lhsT pair-tiles
    [128, 2, 128], each streamed against both 512-wide output halves of
    the un-duplicated fp8 weight, accumulating K_eff = 2048 (hi+lo) into
    two PSUM banks.  The scalar engine copies PSUM -> SBUF bf16 with the
    exact 1/64 scale, and the result DMAs out as bf16 (upcast to fp32 on
    host).  rel err vs the fp32 reference ~3.0e-3.
"""

import numpy as np
import ml_dtypes

B, S, IN, OUT = 8, 8192, 1024, 1024
N_CORES = 8
P = 128
S_BLOCKS = S // P    # 64
K_TILES = IN // P    # 8
LO_SLICES = [4, 5, 6, 7]   # k-slices that get the fp8 lo correction
G = K_TILES + len(LO_SLICES)  # 12 packed k-groups: hi plane + partial lo
EPS = 1e-5

F8 = ml_dtypes.float8_e4m3
BF16 = ml_dtypes.bfloat16

_compiled = None


def _build():
    import concourse.bacc as bacc
    import concourse.mybir as mybir
    import concourse.tile as tile

    F8D = mybir.dt.float8e4
    F32 = mybir.dt.float32
    BF = mybir.dt.bfloat16
    DR = mybir.MatmulPerfMode.DoubleRow

    nc = bacc.Bacc()
    xp = nc.declare_dram_parameter("xp", [S, G * P], F8D, isOutput=False)
    wt = nc.declare_dram_parameter("wt", [IN, OUT], F8D, isOutput=False)
    out = nc.declare_dram_parameter("out", [S, OUT], BF, isOutput=True)

    # DR pair schedule: (x-group pair start, weight k-slice pair start).
    # Groups 0..7 are the hi plane (ks = g); groups 8..11 are the lo plane
    # for k-slices LO_SLICES = 4..7.
    pairs = [(0, 0), (2, 2), (4, 4), (6, 6), (8, 4), (10, 6)]

    with tile.TileContext(nc) as tc:
        with (
            tc.tile_pool(name="const", bufs=1) as constp,
            tc.tile_pool(name="xn", bufs=5) as xnp,
            tc.tile_pool(name="outp", bufs=4) as outp,
            tc.tile_pool(name="pso", bufs=8, space="PSUM") as pso,
        ):
            def load_x(b):
                t = xnp.tile([P, G, P], F8D, tag="xn", name=f"xn_{b}")
                nc.sync.dma_start(out=t, in_=xp[b * P:(b + 1) * P, :])
                return t

            # Block 0's x goes first on the sync ring; the fp8 weight
            # streams concurrently on the gpsimd ring as 8 fully contiguous
            # 128 KB chunks (the first matmuls need only ks 0,1), so the
            # first matmul can start right after the framework preamble.
            xt0 = load_x(0)
            wt_sb = constp.tile([P, K_TILES, OUT], F8D)
            for k in range(K_TILES):
                nc.gpsimd.dma_start(
                    out=wt_sb[:, k:k + 1, :],
                    in_=wt[k * P:(k + 1) * P, :],
                )

            for b in range(S_BLOCKS):
                xt = xt0 if b == 0 else load_x(b)

                po = [pso.tile([P, 512], F32, tag="pso", name=f"po{b}_{h}")
                      for h in range(2)]
                # 6 stationary pair-tiles; each serves both output halves
                # back-to-back so weight loads are halved.
                for t, (xg, wk) in enumerate(pairs):
                    lhsT = xt[:, xg:xg + 2, :]
                    for h in range(2):
                        nc.tensor.matmul(
                            po[h],
                            lhsT=lhsT,
                            rhs=wt_sb[:, wk:wk + 2, h * 512:(h + 1) * 512],
                            start=(t == 0),
                            stop=(t == len(pairs) - 1),
                            perf_mode=DR,
                        )

                ob = outp.tile([P, OUT], BF, tag="ob", name=f"ob_{b}")
                for h in range(2):
                    nc.scalar.activation(
                        ob[:, h * 512:(h + 1) * 512],
                        po[h],
                        mybir.ActivationFunctionType.Copy,
                        scale=1.0 / 64.0,
                    )
                # alternate output rings so neither queue builds a backlog
                ring = nc.gpsimd if b % 2 == 0 else nc.sync
                ring.dma_start(out=out[b * P:(b + 1) * P, :], in_=ob)
    nc.finalize()
    return nc


def _get_compiled():
    global _compiled
    if _compiled is None:
        _compiled = _build()
    return _compiled


def quantize_host(weight: np.ndarray):
    """Mirror of the reference ste_quantize, done on host in fp32.

    The mean is computed in float64 then rounded to fp32 so it tracks the
    true mean more closely than any fp32 summation order.
    """
    scale = np.float32(max(np.mean(np.abs(weight), dtype=np.float64), EPS))
    w_t = np.clip(np.round(weight / scale), -1.0, 1.0).astype(np.float32)
    return w_t, scale


def pack_weight(w_t: np.ndarray) -> np.ndarray:
    """Ternary weight [out, in] -> fp8 transposed [in, out]."""
    return np.ascontiguousarray(w_t.T).astype(F8)


def pack_x_core(xc: np.ndarray, c64: np.float32) -> np.ndarray:
    """One core's x [S, IN] fp32 -> packed fp8 hi + partial lo [S, G*P]."""
    xs = xc * c64
    hi = xs.astype(F8)
    lo_cols = np.concatenate(
        [np.arange(sl * P, (sl + 1) * P) for sl in LO_SLICES])
    lo = (xs[:, lo_cols] - hi.astype(np.float32)[:, lo_cols]).astype(F8)
    xp = np.empty((S_BLOCKS, P, G, P), dtype=F8)
    xp[:, :, 0:K_TILES, :] = hi.reshape(
        S_BLOCKS, P, K_TILES, P).transpose(0, 3, 2, 1)
    xp[:, :, K_TILES:G, :] = lo.reshape(
        S_BLOCKS, P, len(LO_SLICES), P).transpose(0, 3, 2, 1)
    return xp.reshape(S, G * P)


def make_in_maps(x: np.ndarray, weight: np.ndarray):
    x = np.asarray(x, dtype=np.float32)
    weight = np.asarray(weight, dtype=np.float32)
    assert x.shape == (B, S, IN) and weight.shape == (OUT, IN)
    w_t, scale = quantize_host(weight)
    wt8 = pack_weight(w_t)
    c64 = np.float32(64.0) * scale
    from concurrent.futures import ThreadPoolExecutor
    with ThreadPoolExecutor(max_workers=N_CORES) as ex:
        xps = list(ex.map(lambda c: pack_x_core(x[c], c64), range(N_CORES)))
    return [{"xp": xps[c], "wt": wt8} for c in range(N_CORES)]


def kernel(x: np.ndarray, weight: np.ndarray) -> np.ndarray:
    from concourse.bass_utils import run_bass_kernel_spmd

    in_maps = make_in_maps(x, weight)
    nc = _get_compiled()
    res = run_bass_kernel_spmd(nc, in_maps, core_ids=list(range(N_CORES)))
    return np.stack(
        [res.results[c]["out"].astype(np.float32) for c in range(N_CORES)],
        axis=0,
    )
